# revision 13
# baseline (speedup 1.0000x reference)
"""Trainium2 Bass kernel for a 12-layer EVA-style ViT encoder (B=16, N=256, D=768).

Sharding: pure data-parallel over batch across 8 NeuronCores (2 images/core).
Per core: feature-major activations [feature, token] (T=512 token columns).

v2 optimizations over baseline:
- bf16 weights (stationary operands): halves LDWEIGHTS + HBM traffic.
- Stage-resident weights, DMA'd one layer (or one quarter-stage) ahead.
- LN affine folds: Q/K/V and W2 matmuls run on the *unnormalized* stream;
  the (x+B)*A affine is recovered with rank-1 (K=1) accumulation matmuls
  for B, and column-scale folds for A (rope sin/cos for Q/K, per-partition
  tensor_scalar for token-major V, post-scale for W2). The inner-LN (lnm)
  apply on 16 hidden tiles disappears entirely.
- Attention in 2 batches of 12 item-heads: denominators accumulate into a
  [12,256] PSUM tile via selector matmuls; ONE reciprocal_approx_fast per
  batch replaces 12 serial 1.7us DVE reciprocals. psS/exp first, psO/psR
  deferred so PE never waits on Scalar's exp.
- bf16 activation tiles feeding matmuls (q/k/eT/o/s/y2/sq).
"""
import sys, types

sys.path.insert(0, '/opt/trn_rl_repo')

import numpy as np
import ml_dtypes

BF16 = ml_dtypes.bfloat16

B, NTOK, DIM, HEADS, HD, DEPTH, HIDDEN = 16, 256, 768, 12, 64, 12, 2048
EPS = 1e-5
NCORES = 8
BPC = B // NCORES          # batch items per core
T = BPC * NTOK             # 512 token columns per core
KD = DIM // 128            # 6
KH = HIDDEN // 128         # 16
SCALE = HD ** -0.5
NIH = BPC * HEADS          # 24 item-heads per core
NB = 8                     # attention batch size (item-heads per softmax batch)

_CACHE = {}


def _install_ntff_shim():
    if "antenv.axon_hooks" in sys.modules:
        return
    m = types.ModuleType("antenv.axon_hooks")
    m._hook = None
    m.set_axon_ntff_profile_hook = lambda h: setattr(m, "_hook", h)
    m.get_axon_ntff_profile_hook = lambda: m._hook
    sys.modules["antenv.axon_hooks"] = m
    try:
        from trn_agent_boot.trn_boot import _ntff_profile_via_ctypes
        m.set_axon_ntff_profile_hook(_ntff_profile_via_ctypes('/opt/axon/libaxon_pjrt.so'))
    except Exception:
        pass


def _build(layers=DEPTH, dbg=False):
    import concourse.bass as bass
    import concourse.mybir as mybir
    import concourse.tile as tile
    from concourse import bacc
    from contextlib import ExitStack

    f32 = mybir.dt.float32
    f32r = mybir.dt.float32r
    bf16 = mybir.dt.bfloat16
    i32 = mybir.dt.int32
    AF = mybir.ActivationFunctionType
    OP = mybir.AluOpType

    nc = bacc.Bacc("TRN2", target_bir_lowering=False, debug=False)

    x_fm = nc.dram_tensor("x_fm", [DIM, T], f32r, kind="ExternalInput")
    WQ = nc.dram_tensor("WQ", [layers, 128, KD * DIM], bf16, kind="ExternalInput")
    WK = nc.dram_tensor("WK", [layers, 128, KD * DIM], bf16, kind="ExternalInput")
    WV = nc.dram_tensor("WV", [layers, 128, KD * DIM], bf16, kind="ExternalInput")
    WO = nc.dram_tensor("WO", [layers, 128, KD * DIM], bf16, kind="ExternalInput")
    # eighth-streamed MLP weights: [L, 128, 8, KD*256] / [L, 128, 8, 2*DIM]
    WG = nc.dram_tensor("WG", [layers, 128, 8, KD * 256], bf16, kind="ExternalInput")
    WX = nc.dram_tensor("WX", [layers, 128, 8, KD * 256], bf16, kind="ExternalInput")
    W2D = nc.dram_tensor("W2D", [layers, 128, 8, 2 * DIM], bf16, kind="ExternalInput")
    FOLD = nc.dram_tensor("FOLD", [layers, 1, 4 * DIM], bf16, kind="ExternalInput")
    SIN = nc.dram_tensor("SIN", [128, T], f32, kind="ExternalInput")
    COS = nc.dram_tensor("COS", [128, T], f32, kind="ExternalInput")
    PERM = nc.dram_tensor("PERM", [128, 128], bf16, kind="ExternalInput")
    SELD = nc.dram_tensor("SELD", [128, NB * NB], bf16, kind="ExternalInput")
    RSEL = nc.dram_tensor("RSEL", [NB, NB * 64], f32r, kind="ExternalInput")
    ONES = nc.dram_tensor("ONES", [1, 128], f32r, kind="ExternalInput")
    out_fm = nc.dram_tensor("out_fm", [DIM, T], f32, kind="ExternalOutput")
    if dbg:
        DY = nc.dram_tensor("DY", [DIM, T], f32, kind="ExternalOutput")
        DQ = nc.dram_tensor("DQ", [DIM, T], f32, kind="ExternalOutput")
        DK = nc.dram_tensor("DK", [DIM, T], f32, kind="ExternalOutput")
        DV = nc.dram_tensor("DV", [512, DIM], f32, kind="ExternalOutput")
        DO = nc.dram_tensor("DO", [DIM, T], f32, kind="ExternalOutput")
        DH = nc.dram_tensor("DH", [DIM, T], f32, kind="ExternalOutput")
        DS = nc.dram_tensor("DS", [HIDDEN, T], f32, kind="ExternalOutput")

    with tile.TileContext(nc) as tc:
        with ExitStack() as ctx:
            ctx.enter_context(nc.allow_low_precision(
                reason="bf16 weights + f32r activations; tolerance 2e-2"))
            const = ctx.enter_context(tc.tile_pool(name="const", bufs=1))
            hp = ctx.enter_context(tc.tile_pool(name="hp", bufs=1))
            yp = ctx.enter_context(tc.tile_pool(name="yp", bufs=1))
            sqp = ctx.enter_context(tc.tile_pool(name="sqp", bufs=1))
            rowp = ctx.enter_context(tc.tile_pool(name="rowp", bufs=1))
            qrawp = ctx.enter_context(tc.tile_pool(name="qrawp", bufs=3))
            ropep = ctx.enter_context(tc.tile_pool(name="ropep", bufs=1))
            rtmp = ctx.enter_context(tc.tile_pool(name="rtmp", bufs=2))
            rt1 = ctx.enter_context(tc.tile_pool(name="rt1", bufs=1))
            vp = ctx.enter_context(tc.tile_pool(name="vp", bufs=1))
            ep = ctx.enter_context(tc.tile_pool(name="ep", bufs=NB))
            recp = ctx.enter_context(tc.tile_pool(name="recp", bufs=2))
            op_ = ctx.enter_context(tc.tile_pool(name="op", bufs=1))
            sp = ctx.enter_context(tc.tile_pool(name="sp", bufs=2))
            snp = ctx.enter_context(tc.tile_pool(name="snp", bufs=1))
            wqkp = ctx.enter_context(tc.tile_pool(name="wqkp", bufs=1))
            wvop = ctx.enter_context(tc.tile_pool(name="wvop", bufs=1))
            wmp = ctx.enter_context(tc.tile_pool(name="wmp", bufs=2))
            foldp = ctx.enter_context(tc.tile_pool(name="foldp", bufs=1))
            PS = ctx.enter_context(tc.tile_pool(name="PS", bufs=1, space="PSUM"))

            def pst(tag, shape=None, dt_=f32, name=None):
                return PS.tile(shape or [128, T], dt_, name=name or f"ps_{tag}_{nc.next_id()}",
                               tag=tag)

            # ---------------- constants ----------------
            ones1 = const.tile([128, 1], bf16)
            nc.vector.memset(ones1, 1.0)
            ones1r = const.tile([128, 1], f32r)
            nc.gpsimd.dma_start(out=ones1r, in_=bass.AP(ONES.ap().tensor, ONES.ap().offset,
                                                        [[0, 128], [1, 1]]))
            onesk = const.tile([1, 128], f32r)
            nc.sync.dma_start(out=onesk, in_=ONES[:, :])
            warm8 = const.tile([128, 8], bf16)
            nc.vector.memset(warm8, 1.0)
            sin_sb = const.tile([128, T], f32)
            nc.sync.dma_start(out=sin_sb, in_=SIN[:, :])
            cos_sb = const.tile([128, T], f32)
            nc.sync.dma_start(out=cos_sb, in_=COS[:, :])
            perm_sb = const.tile([128, 128], bf16)
            nc.sync.dma_start(out=perm_sb, in_=PERM[:, :])
            seld_sb = const.tile([128, NB * NB], bf16)
            nc.sync.dma_start(out=seld_sb, in_=SELD[:, :])
            rsel_sb = const.tile([NB, NB * 64], f32r)
            nc.sync.dma_start(out=rsel_sb, in_=RSEL[:, :])

            # PE warmup (absorbs entry-barrier waits)
            wps = pst("p7", [8, 8], name="warmps")
            nc.tensor.matmul(wps, warm8[:, :], warm8[:, 0:8], start=True, stop=True)

            h = []
            hb = []
            for k in range(KD):
                t_ = hp.tile([128, T], f32r, name=f"h_{k}", tag=f"h{k}")
                nc.sync.dma_start(out=t_, in_=x_fm[128 * k:128 * (k + 1), :])
                h.append(t_)
                b_ = hp.tile([128, T], bf16, name=f"hb_{k}", tag=f"hb{k}")
                nc.scalar.copy(b_[:, :], t_[:, :])
                hb.append(b_)

            def ln_chain(srow, qrow, D, sfx):
                """DVE scalar chain on [1,T] rows -> (arow=rstd f32r, bneg=-mean f32r)."""
                inv = 1.0 / D
                bneg = rowp.tile([1, T], f32r, name=f"bneg{sfx}", tag=f"bneg{sfx[0]}")
                nc.vector.tensor_scalar(bneg[:, :], srow[:, :], -inv, None, op0=OP.mult)
                msq = rowp.tile([1, T], f32, name=f"msq{sfx}", tag="tt")
                nc.vector.tensor_mul(msq[:, :], bneg[:, :], bneg[:, :])
                ve = rowp.tile([1, T], f32, name=f"ve{sfx}", tag="ve")
                nc.vector.scalar_tensor_tensor(ve[:, :], qrow[:, :], inv, msq[:, :],
                                               op0=OP.mult, op1=OP.subtract)
                nc.vector.tensor_scalar(ve[:, :], ve[:, :], EPS, None, op0=OP.add)
                yv = rowp.tile([1, T], i32, name=f"yv{sfx}", tag="yv")
                nc.vector.tensor_scalar(yv[:, :], ve[:, :].bitcast(i32), 1, None,
                                        op0=OP.logical_shift_right)
                nc.vector.tensor_scalar(yv[:, :], yv[:, :], -1, 0x5f3759df,
                                        op0=OP.mult, op1=OP.add)
                yf = yv[:, :].bitcast(f32)
                tt = rowp.tile([1, T], f32, name=f"tt{sfx}", tag="tt")
                uu = rowp.tile([1, T], f32, name=f"uu{sfx}", tag="uu")
                arow = rowp.tile([1, T], f32r, name=f"arow{sfx}", tag=f"arow{sfx[0]}")
                nc.vector.tensor_mul(tt[:, :], yf, yf)
                nc.vector.tensor_mul(uu[:, :], tt[:, :], ve[:, :])
                nc.vector.tensor_scalar(uu[:, :], uu[:, :], -0.5, 1.5,
                                        op0=OP.mult, op1=OP.add)
                nc.vector.tensor_mul(arow[:, :], yf, uu[:, :])
                bneg_bf = rowp.tile([1, T], bf16, name=f"bnegbf{sfx}",
                                    tag=f"bnegbf{sfx[0]}")
                nc.vector.tensor_copy(out=bneg_bf[:, :], in_=bneg[:, :])
                return arow, bneg, bneg_bf

            for l in range(layers):
                # ------------- weight DMAs (execute during previous layer) -------------
                wq_sb = wqkp.tile([128, KD * DIM], bf16, name=f"wq_{l}", tag="wq")
                nc.sync.dma_start(out=wq_sb, in_=WQ[l, :, :])
                wk_sb = wqkp.tile([128, KD * DIM], bf16, name=f"wk_{l}", tag="wk")
                nc.sync.dma_start(out=wk_sb, in_=WK[l, :, :])
                wv_sb = wvop.tile([128, KD * DIM], bf16, name=f"wv_{l}", tag="wv")
                nc.sync.dma_start(out=wv_sb, in_=WV[l, :, :])
                wo_sb = wvop.tile([128, KD * DIM], bf16, name=f"wo_{l}", tag="wo")
                nc.sync.dma_start(out=wo_sb, in_=WO[l, :, :])
                fold_sb = foldp.tile([1, 4 * DIM], bf16, name=f"fold_{l}", tag="fold")
                nc.sync.dma_start(out=fold_sb, in_=FOLD[l, :, :])
                wgq = []
                wxq = []
                w2q = []
                for qq in range(8):
                    wg_ = wmp.tile([128, KD * 256], bf16, name=f"wg_{l}_{qq}", tag="wg")
                    nc.sync.dma_start(out=wg_, in_=WG[l, :, qq, :])
                    wx_ = wmp.tile([128, KD * 256], bf16, name=f"wx_{l}_{qq}", tag="wx")
                    nc.sync.dma_start(out=wx_, in_=WX[l, :, qq, :])
                    w2_ = wmp.tile([128, 2 * DIM], bf16, name=f"w2_{l}_{qq}", tag="w2")
                    nc.sync.dma_start(out=w2_, in_=W2D[l, :, qq, :])
                    wgq.append(wg_)
                    wxq.append(wx_)
                    w2q.append(w2_)
                qrs = fold_sb[:, 0:DIM]
                krs = fold_sb[:, DIM:2 * DIM]
                vrs = fold_sb[:, 2 * DIM:3 * DIM]
                w2rs = fold_sb[:, 3 * DIM:4 * DIM]

                # ---------------- LN1 stats ----------------
                srow = pst("p0", [1, T], name=f"srow1_{l}")
                qrow = pst("p1", [1, T], name=f"qrow1_{l}")
                for k in range(KD):
                    nc.tensor.matmul(srow, ones1r[:, :], h[k][:, :],
                                     start=(k == 0), stop=(k == KD - 1))
                sqs = []
                for k in range(KD):
                    s_ = sqp.tile([128, T], bf16, name=f"sq1_{l}_{k}", tag=f"sq{k % 2}")
                    nc.scalar.activation(s_[:, :], h[k][:, :], AF.Square)
                    sqs.append(s_)
                for k in range(KD):
                    nc.tensor.matmul(qrow, ones1[:, :], sqs[k][:, :],
                                     start=(k == 0), stop=(k == KD - 1))

                # ---------------- Q,K main projections (on raw h) ----------------
                PROJ_TAGS_QK = ["p2", "p3", "p4", "p5", "p6", "p7"]
                psq = [pst(PROJ_TAGS_QK[m], name=f"psq{l}_{m}") for m in range(KD)]
                for k in range(KD):
                    for m in range(KD):
                        nc.tensor.matmul(psq[m], wq_sb[:, k * DIM + 128 * m:k * DIM + 128 * (m + 1)],
                                         hb[k][:, :], start=(k == 0), stop=False)
                # LN1 chain runs on DVE while Q matmuls stream
                arow, bneg, bneg_bf = ln_chain(srow, qrow, DIM, f"a{l}")
                # fold matmuls close the Q groups
                for m in range(KD):
                    nc.tensor.matmul(psq[m], qrs[:, 128 * m:128 * (m + 1)], bneg_bf[:, :],
                                     start=False, stop=True)
                qk_sb = {}
                outs = []
                for m in range(KD):
                    q_ = qrawp.tile([128, T], bf16, name=f"qsb{l}_{m}", tag=f"qr{m % 2}")
                    nc.scalar.copy(q_[:, :], psq[m][:, :])
                    outs.append(q_)
                qk_sb["q"] = outs

                # A1 broadcast + arowT (token-major rstd column)
                A1 = pst("p0", name=f"A1_{l}")
                nc.tensor.matmul(A1, onesk[:, :], arow[:, :], start=True, stop=True)
                psT = pst("p1", [128, BPC * 4], name=f"psT1_{l}")
                for mt in range(4):
                    nc.tensor.matmul(psT[:, 2 * mt:2 * mt + 2], arow[:, 128 * mt:128 * (mt + 1)],
                                     onesk[:, 0:2], start=True, stop=True)
                arowT = rowp.tile([128, BPC * 4], f32, name=f"arowT{l}", tag="arowT")
                nc.scalar.copy(arowT[:, :], psT[:, :])

                psk = [pst(PROJ_TAGS_QK[m], name=f"psk{l}_{m}") for m in range(KD)]
                for k in range(KD):
                    for m in range(KD):
                        nc.tensor.matmul(psk[m], wk_sb[:, k * DIM + 128 * m:k * DIM + 128 * (m + 1)],
                                         hb[k][:, :], start=(k == 0), stop=False)
                for m in range(KD):
                    nc.tensor.matmul(psk[m], krs[:, 128 * m:128 * (m + 1)], bneg_bf[:, :],
                                     start=False, stop=True)
                outs = []
                for m in range(KD):
                    q_ = qrawp.tile([128, T], bf16, name=f"ksb{l}_{m}", tag=f"kr{m % 2}")
                    nc.scalar.copy(q_[:, :], psk[m][:, :])
                    outs.append(q_)
                qk_sb["k"] = outs

                # sinA/cosA: fold LN1 column scale into rope tables
                sinA = rt1.tile([128, T], f32, name=f"sinA{l}", tag="sinA")
                nc.vector.tensor_mul(sinA[:, :], sin_sb[:, :], A1[:, :])
                cosA = rt1.tile([128, T], bf16, name=f"cosA{l}", tag="cosA")
                nc.vector.tensor_mul(cosA[:, :], cos_sb[:, :], A1[:, :])

                # ---------------- V projection (token-major, fold B then scale A) ----------------
                vtm = []
                for mt in range(4):
                    psvA = pst("p2", [128, 384], name=f"psvA{l}_{mt}")
                    psvB = pst("p3", [128, 384], name=f"psvB{l}_{mt}")
                    for k in range(KD):
                        nc.tensor.matmul(psvA, hb[k][:, 128 * mt:128 * (mt + 1)],
                                         wv_sb[:, k * DIM:k * DIM + 384],
                                         start=(k == 0), stop=False)
                    nc.tensor.matmul(psvA, bneg_bf[:, 128 * mt:128 * (mt + 1)],
                                     vrs[:, 0:384], start=False, stop=True)
                    for k in range(KD):
                        nc.tensor.matmul(psvB, hb[k][:, 128 * mt:128 * (mt + 1)],
                                         wv_sb[:, k * DIM + 384:k * DIM + 768],
                                         start=(k == 0), stop=False)
                    nc.tensor.matmul(psvB, bneg_bf[:, 128 * mt:128 * (mt + 1)],
                                     vrs[:, 384:768], start=False, stop=True)
                    v_ = vp.tile([128, 768], bf16, name=f"vtm{l}_{mt}", tag=f"v{mt}")
                    nc.vector.tensor_scalar(v_[:, 0:384], psvA[:, :], arowT[:, 2 * mt:2 * mt + 1],
                                            None, op0=OP.mult)
                    nc.vector.tensor_scalar(v_[:, 384:768], psvB[:, :], arowT[:, 2 * mt:2 * mt + 1],
                                            None, op0=OP.mult)
                    vtm.append(v_)

                # ---------------- RoPE (A1 folded via sinA/cosA) ----------------
                roped = {"q": [], "k": []}
                for m in range(KD):
                    for wi, wname in enumerate(("q", "k")):
                        raw = qk_sb[wname]
                        rot = pst("p4" if wi == 0 else "p5", name=f"rot{wname}{l}_{m}")
                        nc.tensor.matmul(rot, perm_sb[:, :], raw[m][:, :],
                                         start=True, stop=True)
                        t1 = rtmp.tile([128, T], bf16, name=f"t1{wname}{l}_{m}", tag="t1")
                        nc.vector.tensor_mul(t1[:, :], rot[:, :], sinA[:, :])
                        t2 = rtmp.tile([128, T], bf16, name=f"t2{wname}{l}_{m}", tag="t2")
                        eng = nc.gpsimd if wi == 0 else nc.vector
                        eng.tensor_mul(t2[:, :], raw[m][:, :], cosA[:, :])
                        rp = ropep.tile([128, T], bf16, name=f"{wname}p{l}_{m}",
                                        tag=f"{wname}p{m}")
                        nc.vector.tensor_add(rp[:, :], t1[:, :], t2[:, :])
                        roped[wname].append(rp)
                qs, ks = roped["q"], roped["k"]
                if dbg and l == 0:
                    for k in range(KD):
                        nc.sync.dma_start(out=DQ[128 * k:128 * (k + 1), :], in_=qs[k][:, :])
                        nc.sync.dma_start(out=DK[128 * k:128 * (k + 1), :], in_=ks[k][:, :])
                    for mt in range(4):
                        nc.sync.dma_start(out=DV[128 * mt:128 * (mt + 1), :], in_=vtm[mt][:, 0:768])

                # ---------------- attention: 2 batches of 12 item-heads ----------------
                # hh-major job order so o_sb tiles complete early for O-proj
                o_sb = [op_.tile([128, T], bf16, name=f"osb{l}_{m}", tag=f"o{m}")
                        for m in range(KD)]
                jobs = [(i, hh) for hh in range(HEADS) for i in range(BPC)]
                for bb in range(NIH // NB):
                    bjobs = jobs[bb * NB:(bb + 1) * NB]
                    psD = pst("p0", [NB, NTOK], name=f"psD_{l}_{bb}")
                    eTs = []
                    for jb, (i, hh) in enumerate(bjobs):
                        p, off = hh // 2, 64 * (hh % 2)
                        par = jb % 2
                        psS = pst(["p6", "p2"][par], [128, 2 * NTOK],
                                  name=f"psS{l}_{bb}_{jb}")
                        for kt in range(2):
                            nc.tensor.matmul(
                                psS[:, NTOK * kt:NTOK * (kt + 1)],
                                ks[p][off:off + 64,
                                      256 * i + 128 * kt:256 * i + 128 * (kt + 1)],
                                qs[p][off:off + 64, 256 * i:256 * (i + 1)],
                                start=True, stop=True)
                        eT = ep.tile([128, 2 * NTOK], bf16, name=f"eT{l}_{bb}_{jb}",
                                     tag="eT")
                        nc.scalar.activation(eT[:, :], psS[:, :], AF.Exp)
                        eTs.append(eT)
                        for kt in range(2):
                            nc.tensor.matmul(psD, seld_sb[:, NB * jb:NB * (jb + 1)],
                                             eT[:, NTOK * kt:NTOK * (kt + 1)],
                                             start=(jb == 0 and kt == 0),
                                             stop=(jb == NB - 1 and kt == 1),
                                             skip_group_check=True)
                    rec_f = recp.tile([NB, NTOK], f32, name=f"recf{l}_{bb}",
                                      tag="recf", bufs=1)
                    nc.vector.reciprocal_approx_fast(rec_f[:, :], psD[:, :])
                    rec_all = recp.tile([NB, NTOK], f32r, name=f"recall{l}_{bb}",
                                        tag="recall")
                    nc.vector.tensor_copy(out=rec_all[:, :], in_=rec_f[:, :])
                    for jb, (i, hh) in enumerate(bjobs):
                        p, off = hh // 2, 64 * (hh % 2)
                        psO = pst(["p7", "p3", "p5"][jb % 3], [64, NTOK],
                                  name=f"psO{l}_{bb}_{jb}")
                        for kt in range(2):
                            vt = vtm[2 * i + kt]
                            nc.tensor.matmul(psO, vt[:, 64 * hh:64 * hh + 64],
                                             eTs[jb][:, NTOK * kt:NTOK * (kt + 1)],
                                             start=(kt == 0), stop=(kt == 1))
                        psR = pst(["p1", "p4"][jb % 2], [64, NTOK],
                                  name=f"psR{l}_{bb}_{jb}")
                        nc.tensor.matmul(psR, rsel_sb[:, 64 * jb:64 * (jb + 1)],
                                         rec_all[:, :], start=True, stop=True)
                        rec = recp.tile([64, NTOK], f32, name=f"rec{l}_{bb}_{jb}",
                                        tag="rec")
                        nc.vector.tensor_copy(out=rec[:, :], in_=psR[:, :])
                        nc.vector.tensor_mul(
                            o_sb[p][off:off + 64, 256 * i:256 * (i + 1)],
                            psO[:, :], rec[:, :])

                # ---------------- O projection + residual ----------------
                PROJ_TAGS_O = ["p5", "p6", "p7", "p0", "p2", "p3"]
                pss = [pst(PROJ_TAGS_O[m], name=f"psh{l}_{m}") for m in range(KD)]
                for k in range(KD):
                    for m in range(KD):
                        nc.tensor.matmul(pss[m], wo_sb[:, k * DIM + 128 * m:k * DIM + 128 * (m + 1)],
                                         o_sb[k][:, :], start=(k == 0), stop=(k == KD - 1))
                for m in range(KD):
                    nc.vector.tensor_add(h[m][:, :], h[m][:, :], pss[m][:, :])

                if dbg and l == 0:
                    for k in range(KD):
                        nc.sync.dma_start(out=DO[128 * k:128 * (k + 1), :], in_=o_sb[k][:, :])
                        nc.sync.dma_start(out=DH[128 * k:128 * (k + 1), :], in_=h[k][:, :].bitcast(f32))

                # ---------------- LN2 (materialized y2) ----------------
                srow = pst("p1", [1, T], name=f"srow2_{l}")
                qrow = pst("p4", [1, T], name=f"qrow2_{l}")
                for k in range(KD):
                    nc.tensor.matmul(srow, ones1r[:, :], h[k][:, :],
                                     start=(k == 0), stop=(k == KD - 1))
                sqs = []
                for k in range(KD):
                    s_ = sqp.tile([128, T], bf16, name=f"sq2_{l}_{k}", tag=f"sq{k % 2}")
                    nc.scalar.activation(s_[:, :], h[k][:, :], AF.Square)
                    sqs.append(s_)
                for k in range(KD):
                    nc.tensor.matmul(qrow, ones1[:, :], sqs[k][:, :],
                                     start=(k == 0), stop=(k == KD - 1))
                arow, bneg, bneg_bf = ln_chain(srow, qrow, DIM, f"b{l}")
                A2 = pst("p1", name=f"A2_{l}")
                nc.tensor.matmul(A2, onesk[:, :], arow[:, :], start=True, stop=True)
                B2 = pst("p4", name=f"B2_{l}")
                nc.tensor.matmul(B2, onesk[:, :], bneg[:, :], start=True, stop=True)
                y2 = []
                for k in range(KD):
                    tmp = rtmp.tile([128, T], f32, name=f"l2t{l}_{k}", tag="lnt")
                    nc.vector.tensor_add(tmp[:, :], h[k][:, :], B2[:, :])
                    y_ = yp.tile([128, T], bf16, name=f"y2_{l}_{k}", tag=f"y{k}")
                    nc.vector.tensor_mul(y_[:, :], tmp[:, :], A2[:, :])
                    y2.append(y_)
                if dbg and l == 0:
                    for k in range(KD):
                        nc.sync.dma_start(out=DY[128 * k:128 * (k + 1), :], in_=y2[k][:, :])

                # ---------------- MLP G/U + silu + hidden stats ----------------
                srow2 = pst("p6", [1, T], name=f"srowm_{l}")
                qrow2 = pst("p7", [1, T], name=f"qrowm_{l}")
                s_list = []
                for jj in range(KH):
                    qq, jq = jj // 2, jj % 2
                    psG = pst(["p4", "p5"][jj % 2], name=f"psG{l}_{jj}")
                    psU = pst(["p0", "p1"][jj % 2], name=f"psU{l}_{jj}")
                    for k in range(KD):
                        nc.tensor.matmul(psG, wgq[qq][:, k * 256 + 128 * jq:k * 256 + 128 * (jq + 1)],
                                         y2[k][:, :], start=(k == 0), stop=(k == KD - 1))
                    for k in range(KD):
                        nc.tensor.matmul(psU, wxq[qq][:, k * 256 + 128 * jq:k * 256 + 128 * (jq + 1)],
                                         y2[k][:, :], start=(k == 0), stop=(k == KD - 1))
                    th = sp.tile([128, T], f32, name=f"th{l}_{jj}", tag="th")
                    nc.scalar.activation(th[:, :], psG[:, :], AF.Tanh, scale=0.5)
                    uc = sp.tile([128, T], f32, name=f"uc{l}_{jj}", tag="uc")
                    nc.scalar.copy(uc[:, :], psU[:, :])
                    pp = sp.tile([128, T], f32, name=f"pp{l}_{jj}", tag="pp")
                    nc.vector.tensor_mul(pp[:, :], psG[:, :], uc[:, :])
                    s_ = snp.tile([128, T], bf16, name=f"s{l}_{jj}", tag=f"s{jj}")
                    nc.vector.scalar_tensor_tensor(s_[:, :], th[:, :], 1.0, pp[:, :],
                                                   op0=OP.add, op1=OP.mult)
                    sq_ = sqp.tile([128, T], bf16, name=f"ssq{l}_{jj}", tag=f"sq{jj % 2}")
                    nc.scalar.activation(sq_[:, :], s_[:, :], AF.Square)
                    nc.tensor.matmul(srow2, ones1[:, :], s_[:, :],
                                     start=(jj == 0), stop=(jj == KH - 1))
                    nc.tensor.matmul(qrow2, ones1[:, :], sq_[:, :],
                                     start=(jj == 0), stop=(jj == KH - 1))
                    s_list.append(s_)

                if dbg and l == 0:
                    for jj in range(KH):
                        nc.sync.dma_start(out=DS[128 * jj:128 * (jj + 1), :], in_=s_list[jj][:, :])
                arow, bneg, bneg_bf = ln_chain(srow2, qrow2, HIDDEN, f"m{l}")

                # ---------------- W2 on raw s (lnm folded) + residual ----------------
                PROJ_TAGS_M = ["p4", "p5", "p0", "p1", "p2", "p3"]
                pss = [pst(PROJ_TAGS_M[m], name=f"psm{l}_{m}") for m in range(KD)]
                for k in range(KH):
                    qk_, k4 = k // 2, k % 2
                    for m in range(KD):
                        nc.tensor.matmul(pss[m], w2q[qk_][:, k4 * DIM + 128 * m:k4 * DIM + 128 * (m + 1)],
                                         s_list[k][:, :], start=(k == 0), stop=False)
                Am = pst("p6", name=f"Am_{l}")
                nc.tensor.matmul(Am, onesk[:, :], arow[:, :], start=True, stop=True)
                for m in range(KD):
                    nc.tensor.matmul(pss[m], w2rs[:, 128 * m:128 * (m + 1)], bneg_bf[:, :],
                                     start=False, stop=True)
                Am_sb = rt1.tile([128, T], f32, name=f"Amsb{l}", tag="amsb")
                nc.scalar.copy(Am_sb[:, :], Am[:, :])
                for m in range(KD):
                    tmp = rtmp.tile([128, T], f32, name=f"w2t{l}_{m}", tag="lnt")
                    nc.vector.tensor_mul(tmp[:, :], pss[m][:, :], Am_sb[:, :])
                    nc.vector.tensor_add(h[m][:, :], h[m][:, :], tmp[:, :])
                    nb_ = hp.tile([128, T], bf16, name=f"hb2_{l}_{m}", tag=f"hb{m}")
                    nc.gpsimd.tensor_copy(out=nb_[:, :], in_=h[m][:, :])
                    hb[m] = nb_

            for k in range(KD):
                nc.sync.dma_start(out=out_fm[128 * k:128 * (k + 1), :],
                                  in_=h[k][:, :].bitcast(f32))

    nc.compile()
    return nc


def _prep_host(inputs, layers=DEPTH):
    x = np.asarray(inputs['x'], np.float32)
    pos = np.asarray(inputs['pos_embed'], np.float32)
    rope = np.asarray(inputs['rope_emb'], np.float32)
    g = lambda n: np.asarray(inputs[n], np.float32)

    for n in ('bq', 'bv', 'bo', 'b1g', 'b1x', 'b2', 'ln1_b', 'ln2_b', 'lnm_b'):
        assert np.abs(g(n)).max() == 0.0, f"nonzero bias {n} unsupported"

    ln1w, ln2w, lnmw = g('ln1_w'), g('ln2_w'), g('lnm_w')
    wq = g('wq') * ln1w[:, None, :] * SCALE
    wk = g('wk') * ln1w[:, None, :]
    wv = g('wv') * ln1w[:, None, :]
    wo = g('wo')
    w1g = g('w1g') * ln2w[:, None, :]
    w1x = g('w1x') * ln2w[:, None, :]
    w2 = g('w2') * lnmw[:, None, :]

    L = layers
    tr = lambda w: np.ascontiguousarray(w[:L].transpose(0, 2, 1))

    def pack(w_t, kd, width):
        # [L, kd*128, width] -> [L, 128, kd*width] bf16 (partition-major chunks)
        return np.ascontiguousarray(
            w_t.reshape(L, kd, 128, width).transpose(0, 2, 1, 3).reshape(L, 128, kd * width)
        ).astype(BF16)

    WQh = pack(tr(wq), KD, DIM)
    WKh = pack(tr(wk), KD, DIM)
    WVh = pack(tr(wv), KD, DIM)
    WOh = pack(tr(wo), KD, DIM)
    # quarter-streamed MLP weights
    WGh = np.ascontiguousarray(
        tr(w1g).reshape(L, KD, 128, 8, 2, 128).transpose(0, 2, 3, 1, 4, 5)
        .reshape(L, 128, 8, KD * 256)).astype(BF16)
    WXh = np.ascontiguousarray(
        tr(w1x).reshape(L, KD, 128, 8, 2, 128).transpose(0, 2, 3, 1, 4, 5)
        .reshape(L, 128, 8, KD * 256)).astype(BF16)
    W2h = np.ascontiguousarray(
        tr(w2).reshape(L, 8, 2, 128, DIM).transpose(0, 3, 1, 2, 4)
        .reshape(L, 128, 8, 2 * DIM)).astype(BF16)

    # rank-1 fold rows: rowsums over the contraction (input) dim
    FOLDh = np.concatenate([
        wq[:L].sum(-1), wk[:L].sum(-1), wv[:L].sum(-1), w2[:L].sum(-1),
    ], axis=-1).reshape(L, 1, 4 * DIM).astype(BF16)

    sinp = np.ascontiguousarray(rope[:, :HD].T)
    cosp = np.ascontiguousarray(rope[:, HD:].T)
    SINt = np.tile(sinp, (2, BPC)).astype(np.float32)
    COSt = np.tile(cosp, (2, BPC)).astype(np.float32)

    p64 = np.zeros((64, 64), np.float32)
    for i2 in range(32):
        p64[2 * i2 + 1, 2 * i2] = -1.0
        p64[2 * i2, 2 * i2 + 1] = 1.0
    PERMt = np.zeros((128, 128), np.float32)
    PERMt[0:64, 0:64] = p64
    PERMt[64:128, 64:128] = p64

    SELDt = np.zeros((128, NB * NB), np.float32)
    for j in range(NB):
        SELDt[:, NB * j + j] = 1.0
    RSELt = np.zeros((NB, NB * 64), np.float32)
    for j in range(NB):
        RSELt[j, 64 * j:64 * (j + 1)] = 1.0

    xp = x + pos
    in_maps = []
    for c in range(NCORES):
        xc = xp[BPC * c:BPC * (c + 1)].reshape(T, DIM).T
        in_maps.append({
            "x_fm": np.ascontiguousarray(xc),
            "WQ": WQh, "WK": WKh, "WV": WVh, "WO": WOh,
            "WG": WGh, "WX": WXh, "W2D": W2h, "FOLD": FOLDh,
            "ONES": np.ones((1, 128), np.float32),
            "SIN": SINt, "COS": COSt, "PERM": PERMt.astype(BF16),
            "SELD": SELDt.astype(BF16), "RSEL": RSELt,
        })
    return in_maps


def kernel(_layers=DEPTH, _trace=False, _dbg=False, **inputs):
    _install_ntff_shim()
    from concourse import bass_utils
    key = (_layers, _dbg)
    if key not in _CACHE:
        _CACHE[key] = _build(_layers, dbg=_dbg)
    nc = _CACHE[key]
    in_maps = _prep_host(inputs, _layers)
    res = bass_utils.run_bass_kernel_spmd(nc, in_maps, core_ids=list(range(NCORES)),
                                          trace=_trace)
    out = np.empty((B, NTOK, DIM), np.float32)
    for c in range(NCORES):
        o = res.results[c]["out_fm"]
        out[BPC * c:BPC * (c + 1)] = o.T.reshape(BPC, NTOK, DIM)
    kernel.last_exec_ns = res.exec_time_ns
    kernel.last_res = res
    return out


# revision 14
# speedup vs baseline: 8.1594x; 8.1594x over previous
"""Trainium2 Bass kernel for a 12-layer EVA-style ViT encoder (B=16, N=256, D=768).

Sharding: pure data-parallel over batch across 8 NeuronCores (2 images/core).
Per core: feature-major activations [feature, token] (T=512 token columns).

v2 optimizations over baseline:
- bf16 weights (stationary operands): halves LDWEIGHTS + HBM traffic.
- Stage-resident weights, DMA'd one layer (or one quarter-stage) ahead.
- LN affine folds: Q/K/V and W2 matmuls run on the *unnormalized* stream;
  the (x+B)*A affine is recovered with rank-1 (K=1) accumulation matmuls
  for B, and column-scale folds for A (rope sin/cos for Q/K, per-partition
  tensor_scalar for token-major V, post-scale for W2). The inner-LN (lnm)
  apply on 16 hidden tiles disappears entirely.
- Attention in 2 batches of 12 item-heads: denominators accumulate into a
  [12,256] PSUM tile via selector matmuls; ONE reciprocal_approx_fast per
  batch replaces 12 serial 1.7us DVE reciprocals. psS/exp first, psO/psR
  deferred so PE never waits on Scalar's exp.
- bf16 activation tiles feeding matmuls (q/k/eT/o/s/y2/sq).
"""
import sys, types

sys.path.insert(0, '/opt/trn_rl_repo')

import numpy as np
import ml_dtypes

BF16 = ml_dtypes.bfloat16

B, NTOK, DIM, HEADS, HD, DEPTH, HIDDEN = 16, 256, 768, 12, 64, 12, 2048
EPS = 1e-5
NCORES = 8
BPC = B // NCORES          # batch items per core
T = BPC * NTOK             # 512 token columns per core
KD = DIM // 128            # 6
KH = HIDDEN // 128         # 16
SCALE = HD ** -0.5
NIH = BPC * HEADS          # 24 item-heads per core
NB = 8                     # attention batch size (item-heads per softmax batch)

_CACHE = {}


def _install_ntff_shim():
    if "antenv.axon_hooks" in sys.modules:
        return
    m = types.ModuleType("antenv.axon_hooks")
    m._hook = None
    m.set_axon_ntff_profile_hook = lambda h: setattr(m, "_hook", h)
    m.get_axon_ntff_profile_hook = lambda: m._hook
    sys.modules["antenv.axon_hooks"] = m
    try:
        from trn_agent_boot.trn_boot import _ntff_profile_via_ctypes
        m.set_axon_ntff_profile_hook(_ntff_profile_via_ctypes('/opt/axon/libaxon_pjrt.so'))
    except Exception:
        pass


def _build(layers=DEPTH, dbg=False):
    import concourse.bass as bass
    import concourse.mybir as mybir
    import concourse.tile as tile
    from concourse import bacc
    from contextlib import ExitStack

    f32 = mybir.dt.float32
    f32r = mybir.dt.float32r
    bf16 = mybir.dt.bfloat16
    i32 = mybir.dt.int32
    AF = mybir.ActivationFunctionType
    OP = mybir.AluOpType

    nc = bacc.Bacc("TRN2", target_bir_lowering=False, debug=False)

    x_fm = nc.dram_tensor("x_fm", [DIM, T], f32r, kind="ExternalInput")
    WQ = nc.dram_tensor("WQ", [layers, 128, KD * DIM], bf16, kind="ExternalInput")
    WK = nc.dram_tensor("WK", [layers, 128, KD * DIM], bf16, kind="ExternalInput")
    WV = nc.dram_tensor("WV", [layers, 128, KD * DIM], bf16, kind="ExternalInput")
    WO = nc.dram_tensor("WO", [layers, 128, KD * DIM], bf16, kind="ExternalInput")
    # eighth-streamed MLP weights: [L, 128, 8, KD*256] / [L, 128, 8, 2*DIM]
    WG = nc.dram_tensor("WG", [layers, 128, 8, KD * 256], bf16, kind="ExternalInput")
    WX = nc.dram_tensor("WX", [layers, 128, 8, KD * 256], bf16, kind="ExternalInput")
    W2D = nc.dram_tensor("W2D", [layers, 128, 8, 2 * DIM], bf16, kind="ExternalInput")
    FOLD = nc.dram_tensor("FOLD", [layers, 1, 4 * DIM], bf16, kind="ExternalInput")
    SIN = nc.dram_tensor("SIN", [128, T], f32, kind="ExternalInput")
    COS = nc.dram_tensor("COS", [128, T], f32, kind="ExternalInput")
    PERM = nc.dram_tensor("PERM", [128, 128], bf16, kind="ExternalInput")
    SELD = nc.dram_tensor("SELD", [128, NB * NB], bf16, kind="ExternalInput")
    RSEL = nc.dram_tensor("RSEL", [NB, NB * 64], f32r, kind="ExternalInput")
    ONES = nc.dram_tensor("ONES", [1, 128], f32r, kind="ExternalInput")
    out_fm = nc.dram_tensor("out_fm", [DIM, T], f32, kind="ExternalOutput")
    if dbg:
        DY = nc.dram_tensor("DY", [DIM, T], f32, kind="ExternalOutput")
        DQ = nc.dram_tensor("DQ", [DIM, T], f32, kind="ExternalOutput")
        DK = nc.dram_tensor("DK", [DIM, T], f32, kind="ExternalOutput")
        DV = nc.dram_tensor("DV", [512, DIM], f32, kind="ExternalOutput")
        DO = nc.dram_tensor("DO", [DIM, T], f32, kind="ExternalOutput")
        DH = nc.dram_tensor("DH", [DIM, T], f32, kind="ExternalOutput")
        DS = nc.dram_tensor("DS", [HIDDEN, T], f32, kind="ExternalOutput")

    with tile.TileContext(nc) as tc:
        with ExitStack() as ctx:
            ctx.enter_context(nc.allow_low_precision(
                reason="bf16 weights + f32r activations; tolerance 2e-2"))
            const = ctx.enter_context(tc.tile_pool(name="const", bufs=1))
            hp = ctx.enter_context(tc.tile_pool(name="hp", bufs=1))
            yp = ctx.enter_context(tc.tile_pool(name="yp", bufs=1))
            sqp = ctx.enter_context(tc.tile_pool(name="sqp", bufs=1))
            rowp = ctx.enter_context(tc.tile_pool(name="rowp", bufs=1))
            qrawp = ctx.enter_context(tc.tile_pool(name="qrawp", bufs=3))
            ropep = ctx.enter_context(tc.tile_pool(name="ropep", bufs=1))
            rtmp = ctx.enter_context(tc.tile_pool(name="rtmp", bufs=2))
            rt1 = ctx.enter_context(tc.tile_pool(name="rt1", bufs=1))
            vp = ctx.enter_context(tc.tile_pool(name="vp", bufs=1))
            ep = ctx.enter_context(tc.tile_pool(name="ep", bufs=NB))
            recp = ctx.enter_context(tc.tile_pool(name="recp", bufs=2))
            op_ = ctx.enter_context(tc.tile_pool(name="op", bufs=1))
            sp = ctx.enter_context(tc.tile_pool(name="sp", bufs=2))
            snp = ctx.enter_context(tc.tile_pool(name="snp", bufs=1))
            wqkp = ctx.enter_context(tc.tile_pool(name="wqkp", bufs=1))
            wvop = ctx.enter_context(tc.tile_pool(name="wvop", bufs=1))
            wmp = ctx.enter_context(tc.tile_pool(name="wmp", bufs=2))
            foldp = ctx.enter_context(tc.tile_pool(name="foldp", bufs=1))
            PS = ctx.enter_context(tc.tile_pool(name="PS", bufs=1, space="PSUM"))

            def pst(tag, shape=None, dt_=f32, name=None):
                return PS.tile(shape or [128, T], dt_, name=name or f"ps_{tag}_{nc.next_id()}",
                               tag=tag)

            # ---------------- constants ----------------
            ones1 = const.tile([128, 1], bf16)
            nc.vector.memset(ones1, 1.0)
            ones1r = const.tile([128, 1], f32r)
            nc.gpsimd.dma_start(out=ones1r, in_=bass.AP(ONES.ap().tensor, ONES.ap().offset,
                                                        [[0, 128], [1, 1]]))
            onesk = const.tile([1, 128], f32r)
            nc.sync.dma_start(out=onesk, in_=ONES[:, :])
            warm8 = const.tile([128, 8], bf16)
            nc.vector.memset(warm8, 1.0)
            sin_sb = const.tile([128, T], f32)
            nc.sync.dma_start(out=sin_sb, in_=SIN[:, :])
            cos_sb = const.tile([128, T], f32)
            nc.sync.dma_start(out=cos_sb, in_=COS[:, :])
            perm_sb = const.tile([128, 128], bf16)
            nc.sync.dma_start(out=perm_sb, in_=PERM[:, :])
            seld_sb = const.tile([128, NB * NB], bf16)
            nc.sync.dma_start(out=seld_sb, in_=SELD[:, :])
            rsel_sb = const.tile([NB, NB * 64], f32r)
            nc.sync.dma_start(out=rsel_sb, in_=RSEL[:, :])

            # PE warmup (absorbs entry-barrier waits)
            wps = pst("p7", [8, 8], name="warmps")
            nc.tensor.matmul(wps, warm8[:, :], warm8[:, 0:8], start=True, stop=True)

            h = []
            hb = []
            for k in range(KD):
                t_ = hp.tile([128, T], f32r, name=f"h_{k}", tag=f"h{k}")
                nc.sync.dma_start(out=t_, in_=x_fm[128 * k:128 * (k + 1), :])
                h.append(t_)
                b_ = hp.tile([128, T], bf16, name=f"hb_{k}", tag=f"hb{k}")
                nc.scalar.copy(b_[:, :], t_[:, :])
                hb.append(b_)

            def ln_chain(srow, qrow, D, sfx):
                """DVE scalar chain on [1,T] rows -> (arow=rstd f32r, bneg=-mean f32r)."""
                inv = 1.0 / D
                bneg = rowp.tile([1, T], f32r, name=f"bneg{sfx}", tag=f"bneg{sfx[0]}")
                nc.vector.tensor_scalar(bneg[:, :], srow[:, :], -inv, None, op0=OP.mult)
                msq = rowp.tile([1, T], f32, name=f"msq{sfx}", tag="tt")
                nc.vector.tensor_mul(msq[:, :], bneg[:, :], bneg[:, :])
                ve = rowp.tile([1, T], f32, name=f"ve{sfx}", tag="ve")
                nc.vector.scalar_tensor_tensor(ve[:, :], qrow[:, :], inv, msq[:, :],
                                               op0=OP.mult, op1=OP.subtract)
                nc.vector.tensor_scalar(ve[:, :], ve[:, :], EPS, None, op0=OP.add)
                yv = rowp.tile([1, T], i32, name=f"yv{sfx}", tag="yv")
                nc.vector.tensor_scalar(yv[:, :], ve[:, :].bitcast(i32), 1, None,
                                        op0=OP.logical_shift_right)
                nc.vector.tensor_scalar(yv[:, :], yv[:, :], -1, 0x5f3759df,
                                        op0=OP.mult, op1=OP.add)
                yf = yv[:, :].bitcast(f32)
                tt = rowp.tile([1, T], f32, name=f"tt{sfx}", tag="tt")
                uu = rowp.tile([1, T], f32, name=f"uu{sfx}", tag="uu")
                arow = rowp.tile([1, T], f32r, name=f"arow{sfx}", tag=f"arow{sfx[0]}")
                nc.vector.tensor_mul(tt[:, :], yf, yf)
                nc.vector.tensor_mul(uu[:, :], tt[:, :], ve[:, :])
                nc.vector.tensor_scalar(uu[:, :], uu[:, :], -0.5, 1.5,
                                        op0=OP.mult, op1=OP.add)
                nc.vector.tensor_mul(arow[:, :], yf, uu[:, :])
                bneg_bf = rowp.tile([1, T], bf16, name=f"bnegbf{sfx}",
                                    tag=f"bnegbf{sfx[0]}")
                nc.vector.tensor_copy(out=bneg_bf[:, :], in_=bneg[:, :])
                return arow, bneg, bneg_bf

            for l in range(layers):
                # ------------- weight DMAs (execute during previous layer) -------------
                wq_sb = wqkp.tile([128, KD * DIM], bf16, name=f"wq_{l}", tag="wq")
                nc.sync.dma_start(out=wq_sb, in_=WQ[l, :, :])
                wk_sb = wqkp.tile([128, KD * DIM], bf16, name=f"wk_{l}", tag="wk")
                nc.sync.dma_start(out=wk_sb, in_=WK[l, :, :])
                wv_sb = wvop.tile([128, KD * DIM], bf16, name=f"wv_{l}", tag="wv")
                nc.sync.dma_start(out=wv_sb, in_=WV[l, :, :])
                wo_sb = wvop.tile([128, KD * DIM], bf16, name=f"wo_{l}", tag="wo")
                nc.sync.dma_start(out=wo_sb, in_=WO[l, :, :])
                fold_sb = foldp.tile([1, 4 * DIM], bf16, name=f"fold_{l}", tag="fold")
                nc.sync.dma_start(out=fold_sb, in_=FOLD[l, :, :])
                wgq = []
                wxq = []
                w2q = []
                for qq in range(8):
                    wg_ = wmp.tile([128, KD * 256], bf16, name=f"wg_{l}_{qq}", tag="wg")
                    nc.sync.dma_start(out=wg_, in_=WG[l, :, qq, :])
                    wx_ = wmp.tile([128, KD * 256], bf16, name=f"wx_{l}_{qq}", tag="wx")
                    nc.sync.dma_start(out=wx_, in_=WX[l, :, qq, :])
                    w2_ = wmp.tile([128, 2 * DIM], bf16, name=f"w2_{l}_{qq}", tag="w2")
                    nc.sync.dma_start(out=w2_, in_=W2D[l, :, qq, :])
                    wgq.append(wg_)
                    wxq.append(wx_)
                    w2q.append(w2_)
                qrs = fold_sb[:, 0:DIM]
                krs = fold_sb[:, DIM:2 * DIM]
                vrs = fold_sb[:, 2 * DIM:3 * DIM]
                w2rs = fold_sb[:, 3 * DIM:4 * DIM]

                # ---------------- LN1 stats ----------------
                srow = pst("p0", [1, T], name=f"srow1_{l}")
                qrow = pst("p1", [1, T], name=f"qrow1_{l}")
                for k in range(KD):
                    nc.tensor.matmul(srow, ones1r[:, :], h[k][:, :],
                                     start=(k == 0), stop=(k == KD - 1))
                sqs = []
                for k in range(KD):
                    s_ = sqp.tile([128, T], bf16, name=f"sq1_{l}_{k}", tag=f"sq{k % 2}")
                    nc.scalar.activation(s_[:, :], h[k][:, :], AF.Square)
                    sqs.append(s_)
                for k in range(KD):
                    nc.tensor.matmul(qrow, ones1[:, :], sqs[k][:, :],
                                     start=(k == 0), stop=(k == KD - 1))

                # ---------------- Q,K main projections (on raw h) ----------------
                PROJ_TAGS_QK = ["p2", "p3", "p4", "p5", "p6", "p7"]
                psq = [pst(PROJ_TAGS_QK[m], name=f"psq{l}_{m}") for m in range(KD)]
                for k in range(KD):
                    for m in range(KD):
                        nc.tensor.matmul(psq[m], wq_sb[:, k * DIM + 128 * m:k * DIM + 128 * (m + 1)],
                                         hb[k][:, :], start=(k == 0), stop=False)
                # LN1 chain runs on DVE while Q matmuls stream
                arow, bneg, bneg_bf = ln_chain(srow, qrow, DIM, f"a{l}")
                # fold matmuls close the Q groups
                for m in range(KD):
                    nc.tensor.matmul(psq[m], qrs[:, 128 * m:128 * (m + 1)], bneg_bf[:, :],
                                     start=False, stop=True)
                qk_sb = {}
                outs = []
                for m in range(KD):
                    q_ = qrawp.tile([128, T], bf16, name=f"qsb{l}_{m}", tag=f"qr{m % 2}")
                    nc.scalar.copy(q_[:, :], psq[m][:, :])
                    outs.append(q_)
                qk_sb["q"] = outs

                # A1 broadcast + arowT (token-major rstd column)
                A1 = pst("p0", name=f"A1_{l}")
                nc.tensor.matmul(A1, onesk[:, :], arow[:, :], start=True, stop=True)
                psT = pst("p1", [128, BPC * 4], name=f"psT1_{l}")
                for mt in range(4):
                    nc.tensor.matmul(psT[:, 2 * mt:2 * mt + 2], arow[:, 128 * mt:128 * (mt + 1)],
                                     onesk[:, 0:2], start=True, stop=True)
                arowT = rowp.tile([128, BPC * 4], f32, name=f"arowT{l}", tag="arowT")
                nc.scalar.copy(arowT[:, :], psT[:, :])

                psk = [pst(PROJ_TAGS_QK[m], name=f"psk{l}_{m}") for m in range(KD)]
                for k in range(KD):
                    for m in range(KD):
                        nc.tensor.matmul(psk[m], wk_sb[:, k * DIM + 128 * m:k * DIM + 128 * (m + 1)],
                                         hb[k][:, :], start=(k == 0), stop=False)
                for m in range(KD):
                    nc.tensor.matmul(psk[m], krs[:, 128 * m:128 * (m + 1)], bneg_bf[:, :],
                                     start=False, stop=True)
                outs = []
                for m in range(KD):
                    q_ = qrawp.tile([128, T], bf16, name=f"ksb{l}_{m}", tag=f"kr{m % 2}")
                    nc.scalar.copy(q_[:, :], psk[m][:, :])
                    outs.append(q_)
                qk_sb["k"] = outs

                # sinA/cosA: fold LN1 column scale into rope tables
                sinA = rt1.tile([128, T], f32, name=f"sinA{l}", tag="sinA")
                nc.vector.tensor_mul(sinA[:, :], sin_sb[:, :], A1[:, :])
                cosA = rt1.tile([128, T], bf16, name=f"cosA{l}", tag="cosA")
                nc.vector.tensor_mul(cosA[:, :], cos_sb[:, :], A1[:, :])

                # ---------------- V projection (token-major, fold B then scale A) ----------------
                vtm = []
                for mt in range(4):
                    psvA = pst("p2", [128, 384], name=f"psvA{l}_{mt}")
                    psvB = pst("p3", [128, 384], name=f"psvB{l}_{mt}")
                    for k in range(KD):
                        nc.tensor.matmul(psvA, hb[k][:, 128 * mt:128 * (mt + 1)],
                                         wv_sb[:, k * DIM:k * DIM + 384],
                                         start=(k == 0), stop=False)
                    nc.tensor.matmul(psvA, bneg_bf[:, 128 * mt:128 * (mt + 1)],
                                     vrs[:, 0:384], start=False, stop=True)
                    for k in range(KD):
                        nc.tensor.matmul(psvB, hb[k][:, 128 * mt:128 * (mt + 1)],
                                         wv_sb[:, k * DIM + 384:k * DIM + 768],
                                         start=(k == 0), stop=False)
                    nc.tensor.matmul(psvB, bneg_bf[:, 128 * mt:128 * (mt + 1)],
                                     vrs[:, 384:768], start=False, stop=True)
                    v_ = vp.tile([128, 768], bf16, name=f"vtm{l}_{mt}", tag=f"v{mt}")
                    nc.vector.tensor_scalar(v_[:, 0:384], psvA[:, :], arowT[:, 2 * mt:2 * mt + 1],
                                            None, op0=OP.mult)
                    nc.vector.tensor_scalar(v_[:, 384:768], psvB[:, :], arowT[:, 2 * mt:2 * mt + 1],
                                            None, op0=OP.mult)
                    vtm.append(v_)

                # ---------------- RoPE (A1 folded via sinA/cosA) ----------------
                roped = {"q": [], "k": []}
                for m in range(KD):
                    for wi, wname in enumerate(("q", "k")):
                        raw = qk_sb[wname]
                        rot = pst("p4" if wi == 0 else "p5", name=f"rot{wname}{l}_{m}")
                        nc.tensor.matmul(rot, perm_sb[:, :], raw[m][:, :],
                                         start=True, stop=True)
                        t1 = rtmp.tile([128, T], bf16, name=f"t1{wname}{l}_{m}", tag="t1")
                        nc.vector.tensor_mul(t1[:, :], rot[:, :], sinA[:, :])
                        t2 = rtmp.tile([128, T], bf16, name=f"t2{wname}{l}_{m}", tag="t2")
                        eng = nc.gpsimd if wi == 0 else nc.vector
                        eng.tensor_mul(t2[:, :], raw[m][:, :], cosA[:, :])
                        rp = ropep.tile([128, T], bf16, name=f"{wname}p{l}_{m}",
                                        tag=f"{wname}p{m}")
                        nc.vector.tensor_add(rp[:, :], t1[:, :], t2[:, :])
                        roped[wname].append(rp)
                qs, ks = roped["q"], roped["k"]
                if dbg and l == 0:
                    for k in range(KD):
                        nc.sync.dma_start(out=DQ[128 * k:128 * (k + 1), :], in_=qs[k][:, :])
                        nc.sync.dma_start(out=DK[128 * k:128 * (k + 1), :], in_=ks[k][:, :])
                    for mt in range(4):
                        nc.sync.dma_start(out=DV[128 * mt:128 * (mt + 1), :], in_=vtm[mt][:, 0:768])

                # ---------------- attention: 2 batches of 12 item-heads ----------------
                # hh-major job order so o_sb tiles complete early for O-proj
                o_sb = [op_.tile([128, T], bf16, name=f"osb{l}_{m}", tag=f"o{m}")
                        for m in range(KD)]
                jobs = [(i, hh) for hh in range(HEADS) for i in range(BPC)]
                for bb in range(NIH // NB):
                    bjobs = jobs[bb * NB:(bb + 1) * NB]
                    psD = pst("p0", [NB, NTOK], name=f"psD_{l}_{bb}")
                    eTs = []
                    for jb, (i, hh) in enumerate(bjobs):
                        p, off = hh // 2, 64 * (hh % 2)
                        par = jb % 2
                        psS = pst(["p6", "p2"][par], [128, 2 * NTOK],
                                  name=f"psS{l}_{bb}_{jb}")
                        for kt in range(2):
                            nc.tensor.matmul(
                                psS[:, NTOK * kt:NTOK * (kt + 1)],
                                ks[p][off:off + 64,
                                      256 * i + 128 * kt:256 * i + 128 * (kt + 1)],
                                qs[p][off:off + 64, 256 * i:256 * (i + 1)],
                                start=True, stop=True)
                        eT = ep.tile([128, 2 * NTOK], bf16, name=f"eT{l}_{bb}_{jb}",
                                     tag="eT")
                        nc.scalar.activation(eT[:, :], psS[:, :], AF.Exp)
                        eTs.append(eT)
                        for kt in range(2):
                            nc.tensor.matmul(psD, seld_sb[:, NB * jb:NB * (jb + 1)],
                                             eT[:, NTOK * kt:NTOK * (kt + 1)],
                                             start=(jb == 0 and kt == 0),
                                             stop=(jb == NB - 1 and kt == 1),
                                             skip_group_check=True)
                    rec_f = recp.tile([NB, NTOK], f32, name=f"recf{l}_{bb}",
                                      tag="recf", bufs=1)
                    nc.vector.reciprocal_approx_fast(rec_f[:, :], psD[:, :])
                    rec_all = recp.tile([NB, NTOK], f32r, name=f"recall{l}_{bb}",
                                        tag="recall")
                    nc.vector.tensor_copy(out=rec_all[:, :], in_=rec_f[:, :])
                    for jb, (i, hh) in enumerate(bjobs):
                        p, off = hh // 2, 64 * (hh % 2)
                        psO = pst(["p7", "p3", "p5"][jb % 3], [64, NTOK],
                                  name=f"psO{l}_{bb}_{jb}")
                        for kt in range(2):
                            vt = vtm[2 * i + kt]
                            nc.tensor.matmul(psO, vt[:, 64 * hh:64 * hh + 64],
                                             eTs[jb][:, NTOK * kt:NTOK * (kt + 1)],
                                             start=(kt == 0), stop=(kt == 1))
                        psR = pst(["p1", "p4"][jb % 2], [64, NTOK],
                                  name=f"psR{l}_{bb}_{jb}")
                        nc.tensor.matmul(psR, rsel_sb[:, 64 * jb:64 * (jb + 1)],
                                         rec_all[:, :], start=True, stop=True)
                        rec = recp.tile([64, NTOK], f32, name=f"rec{l}_{bb}_{jb}",
                                        tag="rec")
                        nc.vector.tensor_copy(out=rec[:, :], in_=psR[:, :])
                        nc.vector.tensor_mul(
                            o_sb[p][off:off + 64, 256 * i:256 * (i + 1)],
                            psO[:, :], rec[:, :])

                # ---------------- O projection + residual ----------------
                PROJ_TAGS_O = ["p5", "p6", "p7", "p0", "p2", "p3"]
                pss = [pst(PROJ_TAGS_O[m], name=f"psh{l}_{m}") for m in range(KD)]
                for k in range(KD):
                    for m in range(KD):
                        nc.tensor.matmul(pss[m], wo_sb[:, k * DIM + 128 * m:k * DIM + 128 * (m + 1)],
                                         o_sb[k][:, :], start=(k == 0), stop=(k == KD - 1))
                for m in range(KD):
                    nc.vector.tensor_add(h[m][:, :], h[m][:, :], pss[m][:, :])

                if dbg and l == 0:
                    for k in range(KD):
                        nc.sync.dma_start(out=DO[128 * k:128 * (k + 1), :], in_=o_sb[k][:, :])
                        nc.sync.dma_start(out=DH[128 * k:128 * (k + 1), :], in_=h[k][:, :].bitcast(f32))

                # ---------------- LN2 (materialized y2) ----------------
                srow = pst("p1", [1, T], name=f"srow2_{l}")
                qrow = pst("p4", [1, T], name=f"qrow2_{l}")
                for k in range(KD):
                    nc.tensor.matmul(srow, ones1r[:, :], h[k][:, :],
                                     start=(k == 0), stop=(k == KD - 1))
                sqs = []
                for k in range(KD):
                    s_ = sqp.tile([128, T], bf16, name=f"sq2_{l}_{k}", tag=f"sq{k % 2}")
                    nc.scalar.activation(s_[:, :], h[k][:, :], AF.Square)
                    sqs.append(s_)
                for k in range(KD):
                    nc.tensor.matmul(qrow, ones1[:, :], sqs[k][:, :],
                                     start=(k == 0), stop=(k == KD - 1))
                arow, bneg, bneg_bf = ln_chain(srow, qrow, DIM, f"b{l}")
                A2 = pst("p1", name=f"A2_{l}")
                nc.tensor.matmul(A2, onesk[:, :], arow[:, :], start=True, stop=True)
                B2 = pst("p4", name=f"B2_{l}")
                nc.tensor.matmul(B2, onesk[:, :], bneg[:, :], start=True, stop=True)
                y2 = []
                for k in range(KD):
                    tmp = rtmp.tile([128, T], f32, name=f"l2t{l}_{k}", tag="lnt")
                    nc.vector.tensor_add(tmp[:, :], h[k][:, :], B2[:, :])
                    y_ = yp.tile([128, T], bf16, name=f"y2_{l}_{k}", tag=f"y{k}")
                    nc.vector.tensor_mul(y_[:, :], tmp[:, :], A2[:, :])
                    y2.append(y_)
                if dbg and l == 0:
                    for k in range(KD):
                        nc.sync.dma_start(out=DY[128 * k:128 * (k + 1), :], in_=y2[k][:, :])

                # ---------------- MLP G/U + silu + hidden stats ----------------
                srow2 = pst("p6", [1, T], name=f"srowm_{l}")
                qrow2 = pst("p7", [1, T], name=f"qrowm_{l}")
                s_list = []
                for jj in range(KH):
                    qq, jq = jj // 2, jj % 2
                    psG = pst(["p4", "p5"][jj % 2], name=f"psG{l}_{jj}")
                    psU = pst(["p0", "p1"][jj % 2], name=f"psU{l}_{jj}")
                    for k in range(KD):
                        nc.tensor.matmul(psG, wgq[qq][:, k * 256 + 128 * jq:k * 256 + 128 * (jq + 1)],
                                         y2[k][:, :], start=(k == 0), stop=(k == KD - 1))
                    for k in range(KD):
                        nc.tensor.matmul(psU, wxq[qq][:, k * 256 + 128 * jq:k * 256 + 128 * (jq + 1)],
                                         y2[k][:, :], start=(k == 0), stop=(k == KD - 1))
                    th = sp.tile([128, T], f32, name=f"th{l}_{jj}", tag="th")
                    nc.scalar.activation(th[:, :], psG[:, :], AF.Tanh, scale=0.5)
                    uc = sp.tile([128, T], f32, name=f"uc{l}_{jj}", tag="uc")
                    nc.scalar.copy(uc[:, :], psU[:, :])
                    pp = sp.tile([128, T], f32, name=f"pp{l}_{jj}", tag="pp")
                    nc.vector.tensor_mul(pp[:, :], psG[:, :], uc[:, :])
                    s_ = snp.tile([128, T], bf16, name=f"s{l}_{jj}", tag=f"s{jj}")
                    nc.vector.scalar_tensor_tensor(s_[:, :], th[:, :], 1.0, pp[:, :],
                                                   op0=OP.add, op1=OP.mult)
                    sq_ = sqp.tile([128, T], bf16, name=f"ssq{l}_{jj}", tag=f"sq{jj % 2}")
                    nc.scalar.activation(sq_[:, :], s_[:, :], AF.Square)
                    nc.tensor.matmul(srow2, ones1[:, :], s_[:, :],
                                     start=(jj == 0), stop=(jj == KH - 1))
                    nc.tensor.matmul(qrow2, ones1[:, :], sq_[:, :],
                                     start=(jj == 0), stop=(jj == KH - 1))
                    s_list.append(s_)

                if dbg and l == 0:
                    for jj in range(KH):
                        nc.sync.dma_start(out=DS[128 * jj:128 * (jj + 1), :], in_=s_list[jj][:, :])
                arow, bneg, bneg_bf = ln_chain(srow2, qrow2, HIDDEN, f"m{l}")

                # ---------------- W2 on raw s (lnm folded) + residual ----------------
                PROJ_TAGS_M = ["p4", "p5", "p0", "p1", "p2", "p3"]
                pss = [pst(PROJ_TAGS_M[m], name=f"psm{l}_{m}") for m in range(KD)]
                for k in range(KH):
                    qk_, k4 = k // 2, k % 2
                    for m in range(KD):
                        nc.tensor.matmul(pss[m], w2q[qk_][:, k4 * DIM + 128 * m:k4 * DIM + 128 * (m + 1)],
                                         s_list[k][:, :], start=(k == 0), stop=False)
                Am = pst("p6", name=f"Am_{l}")
                nc.tensor.matmul(Am, onesk[:, :], arow[:, :], start=True, stop=True)
                for m in range(KD):
                    nc.tensor.matmul(pss[m], w2rs[:, 128 * m:128 * (m + 1)], bneg_bf[:, :],
                                     start=False, stop=True)
                Am_sb = rt1.tile([128, T], f32, name=f"Amsb{l}", tag="amsb")
                nc.scalar.copy(Am_sb[:, :], Am[:, :])
                for m in range(KD):
                    tmp = rtmp.tile([128, T], f32, name=f"w2t{l}_{m}", tag="lnt")
                    nc.vector.tensor_mul(tmp[:, :], pss[m][:, :], Am_sb[:, :])
                    nc.vector.tensor_add(h[m][:, :], h[m][:, :], tmp[:, :])
                    nb_ = hp.tile([128, T], bf16, name=f"hb2_{l}_{m}", tag=f"hb{m}")
                    if m % 3 == 0:
                        nc.gpsimd.tensor_copy(out=nb_[:, :], in_=h[m][:, :])
                    elif m % 3 == 1:
                        nc.scalar.copy(nb_[:, :], h[m][:, :])
                    else:
                        nc.vector.tensor_copy(out=nb_[:, :], in_=h[m][:, :])
                    hb[m] = nb_

            for k in range(KD):
                nc.sync.dma_start(out=out_fm[128 * k:128 * (k + 1), :],
                                  in_=h[k][:, :].bitcast(f32))

    nc.compile()
    return nc


def _prep_host(inputs, layers=DEPTH):
    x = np.asarray(inputs['x'], np.float32)
    pos = np.asarray(inputs['pos_embed'], np.float32)
    rope = np.asarray(inputs['rope_emb'], np.float32)
    g = lambda n: np.asarray(inputs[n], np.float32)

    for n in ('bq', 'bv', 'bo', 'b1g', 'b1x', 'b2', 'ln1_b', 'ln2_b', 'lnm_b'):
        assert np.abs(g(n)).max() == 0.0, f"nonzero bias {n} unsupported"

    ln1w, ln2w, lnmw = g('ln1_w'), g('ln2_w'), g('lnm_w')
    wq = g('wq') * ln1w[:, None, :] * SCALE
    wk = g('wk') * ln1w[:, None, :]
    wv = g('wv') * ln1w[:, None, :]
    wo = g('wo')
    w1g = g('w1g') * ln2w[:, None, :]
    w1x = g('w1x') * ln2w[:, None, :]
    w2 = g('w2') * lnmw[:, None, :]

    L = layers
    tr = lambda w: np.ascontiguousarray(w[:L].transpose(0, 2, 1))

    def pack(w_t, kd, width):
        # [L, kd*128, width] -> [L, 128, kd*width] bf16 (partition-major chunks)
        return np.ascontiguousarray(
            w_t.reshape(L, kd, 128, width).transpose(0, 2, 1, 3).reshape(L, 128, kd * width)
        ).astype(BF16)

    WQh = pack(tr(wq), KD, DIM)
    WKh = pack(tr(wk), KD, DIM)
    WVh = pack(tr(wv), KD, DIM)
    WOh = pack(tr(wo), KD, DIM)
    # quarter-streamed MLP weights
    WGh = np.ascontiguousarray(
        tr(w1g).reshape(L, KD, 128, 8, 2, 128).transpose(0, 2, 3, 1, 4, 5)
        .reshape(L, 128, 8, KD * 256)).astype(BF16)
    WXh = np.ascontiguousarray(
        tr(w1x).reshape(L, KD, 128, 8, 2, 128).transpose(0, 2, 3, 1, 4, 5)
        .reshape(L, 128, 8, KD * 256)).astype(BF16)
    W2h = np.ascontiguousarray(
        tr(w2).reshape(L, 8, 2, 128, DIM).transpose(0, 3, 1, 2, 4)
        .reshape(L, 128, 8, 2 * DIM)).astype(BF16)

    # rank-1 fold rows: rowsums over the contraction (input) dim
    FOLDh = np.concatenate([
        wq[:L].sum(-1), wk[:L].sum(-1), wv[:L].sum(-1), w2[:L].sum(-1),
    ], axis=-1).reshape(L, 1, 4 * DIM).astype(BF16)

    sinp = np.ascontiguousarray(rope[:, :HD].T)
    cosp = np.ascontiguousarray(rope[:, HD:].T)
    SINt = np.tile(sinp, (2, BPC)).astype(np.float32)
    COSt = np.tile(cosp, (2, BPC)).astype(np.float32)

    p64 = np.zeros((64, 64), np.float32)
    for i2 in range(32):
        p64[2 * i2 + 1, 2 * i2] = -1.0
        p64[2 * i2, 2 * i2 + 1] = 1.0
    PERMt = np.zeros((128, 128), np.float32)
    PERMt[0:64, 0:64] = p64
    PERMt[64:128, 64:128] = p64

    SELDt = np.zeros((128, NB * NB), np.float32)
    for j in range(NB):
        SELDt[:, NB * j + j] = 1.0
    RSELt = np.zeros((NB, NB * 64), np.float32)
    for j in range(NB):
        RSELt[j, 64 * j:64 * (j + 1)] = 1.0

    xp = x + pos
    in_maps = []
    for c in range(NCORES):
        xc = xp[BPC * c:BPC * (c + 1)].reshape(T, DIM).T
        in_maps.append({
            "x_fm": np.ascontiguousarray(xc),
            "WQ": WQh, "WK": WKh, "WV": WVh, "WO": WOh,
            "WG": WGh, "WX": WXh, "W2D": W2h, "FOLD": FOLDh,
            "ONES": np.ones((1, 128), np.float32),
            "SIN": SINt, "COS": COSt, "PERM": PERMt.astype(BF16),
            "SELD": SELDt.astype(BF16), "RSEL": RSELt,
        })
    return in_maps


def kernel(_layers=DEPTH, _trace=False, _dbg=False, **inputs):
    _install_ntff_shim()
    from concourse import bass_utils
    key = (_layers, _dbg)
    if key not in _CACHE:
        _CACHE[key] = _build(_layers, dbg=_dbg)
    nc = _CACHE[key]
    in_maps = _prep_host(inputs, _layers)
    res = bass_utils.run_bass_kernel_spmd(nc, in_maps, core_ids=list(range(NCORES)),
                                          trace=_trace)
    out = np.empty((B, NTOK, DIM), np.float32)
    for c in range(NCORES):
        o = res.results[c]["out_fm"]
        out[BPC * c:BPC * (c + 1)] = o.T.reshape(BPC, NTOK, DIM)
    kernel.last_exec_ns = res.exec_time_ns
    kernel.last_res = res
    return out
